# revision 7
# baseline (speedup 1.0000x reference)
"""Trainium2 Bass kernel for nn_Network_84026740179249 (tree-LSTM message passing).

Strategy (8 NeuronCores, data-parallel over the node axis; 2048 nodes/core):
  - Host (fp32 numpy): squeeze linears + train-mode BatchNorm folded into per-level
    LSTM inputs; features pre-transposed per core into feature-major fp16; LSTM
    weights packed with gate columns reordered to [i,f,o,g]; bias folded in via a
    ones-row; 0.5 child-averaging folded into Whh and the stored c values.
  - Device, per level: parent h||c rows fetched node-major with 8 dma_gather ops
    (512 rows x 512B each) spread over 4 SWDGE queues from a per-level AllGathered
    state table [8*2049, 256] fp16 (zero row per rank shard handles mapping==0).
    Child pairs summed on VectorE; the h half transposed to feature-major via 16
    TensorE transposes; gates accumulate node-major in PSUM ([i,f,o,g] x 4-chunk
    groups); ScalarE sigmoid/tanh epilogues; VectorE c/h updates; writeback +
    AllGather feeds the next level.
  - Host: final 20->1 MLP head + sigmoid in fp32.
"""

import os
import sys

import numpy as np

_NOAG = os.environ.get("K_NOAG", "0") == "1"
_NOGATHER = os.environ.get("K_NOGATHER", "0") == "1"
_NOMM = os.environ.get("K_NOMM", "0") == "1"

sys.path.insert(0, "/opt/trn_rl_repo")

from concourse import bass, mybir, bacc, tile  # noqa: E402
from concourse import bass_utils  # noqa: E402

F32 = mybir.dt.float32
F16 = mybir.dt.float16
I16 = mybir.dt.int16

L = 24
N = 16384
H = 128
NCORE = 8
SH = N // NCORE          # 2048 nodes per core
CH = SH // 128           # 16 node chunks of 128
ROWS = SH + 1            # per-rank table rows (zero row + shard)
NT = NCORE * ROWS        # 16392 table rows
EPS = 1e-5
SLOPE = 0.01

_CACHE = {}


def _build_program():
    nc = bacc.Bacc("TRN2", target_bir_lowering=False, debug=False,
                   num_devices=NCORE, num_swdge_queues=4)

    xa_d = nc.dram_tensor("xa", [L, 128, SH], F16, kind="ExternalInput").ap()
    xb_d = nc.dram_tensor("xb", [L, 9, SH], F16, kind="ExternalInput").ap()
    idx_d = nc.dram_tensor("idx", [L - 1, 128, 256], I16, kind="ExternalInput").ap()
    wa_d = nc.dram_tensor("wa", [128, 512], F16, kind="ExternalInput").ap()
    wb_d = nc.dram_tensor("wb", [9, 512], F16, kind="ExternalInput").ap()
    whh_d = nc.dram_tensor("whh", [128, 512], F16, kind="ExternalInput").ap()
    ident_d = nc.dram_tensor("ident", [128, 128], F16, kind="ExternalInput").ap()
    hout_d = nc.dram_tensor("hout", [SH, H], F32, kind="ExternalOutput").ap()

    with tile.TileContext(nc) as tc:
        with (
            tc.tile_pool(name="const", bufs=1) as cp,
            tc.tile_pool(name="sb", bufs=1) as sb,
            tc.tile_pool(name="ps", bufs=1, space="PSUM") as ps,
            tc.tile_pool(name="dram", bufs=1, space="DRAM") as dram,
        ):
            wa = cp.tile([128, 512], F16)
            wb = cp.tile([9, 512], F16)
            whh = cp.tile([128, 512], F16)
            ident = cp.tile([128, 128], F16)
            nc.sync.dma_start(wa[:], wa_d)
            nc.sync.dma_start(wb[:], wb_d)
            nc.sync.dma_start(whh[:], whh_d)
            nc.sync.dma_start(ident[:], ident_d)
            zrow = cp.tile([1, 256], F16)
            nc.vector.memset(zrow[:], 0.0)

            tabs = [
                dram.tile([NT, 256], F16, addr_space="Local" if _NOAG else "Shared", name=f"tab{s}")
                for s in range(L - 1)
            ]

            Sig = mybir.ActivationFunctionType.Sigmoid
            Tanh = mybir.ActivationFunctionType.Tanh
            TT = nc.vector.tensor_tensor
            MUL = mybir.AluOpType.mult
            ADD = mybir.AluOpType.add

            for s in range(L):
                xa = sb.tile([128, SH], F16, name=f"xa{s}", tag="xa", bufs=3)
                xb = sb.tile([9, SH], F16, name=f"xb{s}", tag="xb", bufs=3)
                nc.sync.dma_start(xa[:], xa_d[s])
                nc.sync.dma_start(xb[:], xb_d[s])

                if s > 0 and not _NOGATHER:
                    idx = sb.tile([128, 256], I16, name=f"idx{s}", tag="idx", bufs=3)
                    nc.sync.dma_start(idx[:], idx_d[s - 1])
                    tab = tabs[s - 1]

                    # 8 full-row gathers (512 rows x 256 f16), paired m0/m1 issue
                    # order so sums can start early; 4 SWDGE queues in parallel.
                    graw = [None] * 8
                    qq = 0
                    for k in (0, 4, 1, 5, 2, 6, 3, 7):
                        g = sb.tile([128, 4, 256], F16, name=f"graw{s}_{k}",
                                    tag=f"graw{k}", bufs=2)
                        nc.gpsimd.dma_gather(
                            out_ap=g[:], in_ap=tab[:],
                            idxs_ap=idx[:, k * 32:(k + 1) * 32],
                            num_idxs=512, num_idxs_reg=512,
                            elem_size=256, elem_step=256, transpose=False,
                            queue_num=qq % 4,
                        )
                        qq += 1
                        graw[k] = g
                    gsum = sb.tile([128, CH, 256], F16, name=f"gsum{s}",
                                   tag="gsum", bufs=2)
                    for k in range(4):
                        TT(out=gsum[:, 4 * k:4 * k + 4, :], in0=graw[k][:],
                           in1=graw[k + 4][:], op=ADD)

                    # h-half -> feature-major via 16 PE transposes, one copy out
                    tp = ps.tile([128, 4, 512], F16, name=f"tp{s}", tag="gates",
                                 bufs=2, space="PSUM")
                    for t in range(CH):
                        nc.tensor.transpose(
                            out=tp[:, t // 4, (t % 4) * 128:(t % 4 + 1) * 128],
                            in_=gsum[:, t, 0:128], identity=ident[:],
                        )
                    ghT = sb.tile([128, SH], F16, name=f"ghT{s}", tag="ghT", bufs=2)
                    nc.vector.tensor_copy(
                        ghT[:].rearrange("p (a b) -> p a b", a=4), tp[:])

                if s < L - 1:
                    wbt = sb.tile([128, CH, 256], F16, name=f"wbt{s}", tag="wbt", bufs=2)
                else:
                    hlast = sb.tile([128, CH, 128], F32, name="hlast")

                # gate groups of 4 node-chunks; gate cols reordered [i|f|o|g]
                for g4 in range(4):
                    psg = ps.tile([128, 4, 512], F32, name=f"ps{s}_{g4}",
                                  tag="gates", bufs=2, space="PSUM")
                    for cc in range(4):
                        t = 4 * g4 + cc
                        nc.tensor.matmul(
                            psg[:, cc, :], lhsT=xa[:, t * 128:(t + 1) * 128],
                            rhs=wa[:], start=True, stop=False,
                        )
                        if not _NOMM:
                            nc.tensor.matmul(
                                psg[:, cc, :], lhsT=xb[:, t * 128:(t + 1) * 128],
                                rhs=wb[:], start=False, stop=(s == 0),
                            )
                        if s > 0 and not _NOGATHER:
                            nc.tensor.matmul(
                                psg[:, cc, :], lhsT=ghT[:, t * 128:(t + 1) * 128],
                                rhs=whh[:], start=False, stop=True,
                            )
                    sig = sb.tile([128, 4, 512], F16, name=f"sig{s}_{g4}",
                                  tag="sig", bufs=2)
                    nc.scalar.activation(sig[:, :, 0:384], psg[:, :, 0:384], Sig)
                    nc.scalar.activation(sig[:, :, 384:512], psg[:, :, 384:512], Tanh)

                    t2 = sb.tile([128, 4, 128], F16, name=f"t2{s}_{g4}", tag="t2", bufs=2)
                    cn = sb.tile([128, 4, 128], F16, name=f"cn{s}_{g4}", tag="cn", bufs=2)
                    TT(out=t2[:], in0=sig[:, :, 0:128], in1=sig[:, :, 384:512], op=MUL)
                    if s > 0 and not _NOGATHER:
                        t1 = sb.tile([128, 4, 128], F16, name=f"t1{s}_{g4}",
                                     tag="t1", bufs=2)
                        TT(out=t1[:], in0=sig[:, :, 128:256],
                           in1=gsum[:, 4 * g4:4 * g4 + 4, 128:256], op=MUL)
                        TT(out=cn[:], in0=t1[:], in1=t2[:], op=ADD)
                    else:
                        nc.vector.tensor_copy(cn[:], t2[:])
                    tcg = sb.tile([128, 4, 128], F16, name=f"tc{s}_{g4}", tag="tcg", bufs=2)
                    nc.scalar.activation(tcg[:], cn[:], Tanh)
                    if s < L - 1:
                        TT(out=wbt[:, 4 * g4:4 * g4 + 4, 0:128],
                           in0=sig[:, :, 256:384], in1=tcg[:], op=MUL)
                        nc.vector.tensor_scalar_mul(
                            wbt[:, 4 * g4:4 * g4 + 4, 128:256], cn[:], 0.5)
                    else:
                        TT(out=hlast[:, 4 * g4:4 * g4 + 4, :],
                           in0=sig[:, :, 256:384], in1=tcg[:], op=MUL)

                if s < L - 1:
                    bounce = dram.tile([ROWS, 256], F16, name=f"bounce{s}",
                                       tag="bounce", bufs=2)
                    nc.sync.dma_start(bounce[0:1, :], zrow[:])
                    nc.sync.dma_start(
                        bounce[1:ROWS, :].rearrange("(p t) f -> p t f", p=128), wbt[:])
                    if _NOAG:
                        nc.sync.dma_start(
                            tabs[s][0:SH, :].rearrange("(p t) f -> p t f", p=128),
                            wbt[:])
                        nc.sync.dma_start(
                            tabs[s][SH:2*SH, :].rearrange("(p t) f -> p t f", p=128),
                            wbt[:])
                    else:
                        nc.gpsimd.collective_compute(
                            "AllGather", mybir.AluOpType.bypass,
                            replica_groups=[list(range(NCORE))],
                            ins=[bounce[:].opt()], outs=[tabs[s][:].opt()],
                        )
                else:
                    nc.sync.dma_start(
                        hout_d.rearrange("(p t) f -> p t f", p=128), hlast[:])

    nc.compile()
    return nc


def _leaky(x):
    return np.where(x > 0, x, SLOPE * x).astype(np.float32)


def _bn(x, g, b):
    m = x.mean(axis=0)
    v = x.var(axis=0)
    return (x - m) / np.sqrt(v + EPS) * g + b


# gate-column reorder: torch [i f g o] -> device [i f o g]
_PERM = np.concatenate([np.arange(0, 256), np.arange(384, 512), np.arange(256, 384)])


def kernel(op_pad, attr_pad, filter_pad, output_pad, mapping_pad, batch_size,
           Wf, bf, Wo, bo, g1, beta1, g2, beta2, Wih, Whh, bih, bhh, W1, b1, W2, b2):
    f = lambda a: np.asarray(a, dtype=np.float32)
    op_pad, attr_pad = f(op_pad), f(attr_pad)
    filter_pad, output_pad = f(filter_pad), f(output_pad)
    mapping = np.asarray(mapping_pad, dtype=np.int64)
    bs = int(batch_size)
    Wf, bf, Wo, bo = f(Wf), f(bf), f(Wo), f(bo)
    g1, beta1, g2, beta2 = f(g1), f(beta1), f(g2), f(beta2)
    Wih, Whh, bih, bhh = f(Wih), f(Whh), f(bih), f(bhh)
    W1, b1, W2, b2 = f(W1), f(b1), f(W2), f(b2)

    # ---- host: squeeze + batchnorm (training-mode batch stats), fp32
    fd = _bn(_leaky(filter_pad.reshape(L * N, -1) @ Wf.T + bf), g1, beta1)
    od = _bn(_leaky(output_pad.reshape(L * N, -1) @ Wo.T + bo), g2, beta2)
    x_cat = np.concatenate(
        [op_pad.reshape(L * N, -1), attr_pad.reshape(L * N, -1), fd, od], axis=1
    ).reshape(L, N, 136)  # [op16 | attr56 | fd32 | od32]

    # processing order: step s handles tree level L-1-s
    x_proc = x_cat[::-1]
    map_proc = mapping[::-1]

    # physical layout: core c local j -> table row c*ROWS + 1 + (j%128)*CH + j//128
    jloc = np.arange(SH)
    phys_local = (jloc % 128) * CH + jloc // 128
    node = np.arange(N)
    tab_row = (node // SH) * ROWS + 1 + phys_local[node % SH]

    WihP = Wih[_PERM]  # reorder gate rows to [i f o g]
    WhhP = Whh[_PERM]
    biasP = (bih + bhh)[_PERM]

    in_maps = []
    for c in range(NCORE):
        cs, ce = c * SH, (c + 1) * SH
        xs = x_proc[:, cs:ce, :]
        xaT = np.ascontiguousarray(xs[:, :, 0:128].transpose(0, 2, 1)).astype(np.float16)
        xbT = np.empty((L, 9, SH), np.float16)
        xbT[:, 0:8, :] = xs[:, :, 128:136].transpose(0, 2, 1)
        xbT[:, 8, :] = 1.0

        m = map_proc[1:, cs:ce, :]
        gidx = np.where(m == 0, 0, tab_row[np.maximum(m - 1, 0)]).astype(np.int16)
        vec = np.concatenate([gidx[:, :, 0], gidx[:, :, 1]], axis=1)  # [L-1, 2*SH]
        wrapped = vec.reshape(L - 1, 8, 32, 16).transpose(0, 1, 3, 2)  # [L-1,8,16,32]
        idx_in = np.tile(wrapped, (1, 1, 8, 1))                        # [L-1,8,128,32]
        idx_in = np.concatenate([idx_in[:, k] for k in range(8)], axis=2)

        in_maps.append({
            "xa": xaT,
            "xb": xbT,
            "idx": np.ascontiguousarray(idx_in),
            "wa": WihP[:, 0:128].T.astype(np.float16).copy(),
            "wb": np.concatenate(
                [WihP[:, 128:136].T, biasP[None, :]], axis=0).astype(np.float16).copy(),
            "whh": (0.5 * WhhP.T).astype(np.float16).copy(),
            "ident": np.eye(128, dtype=np.float16),
        })

    if "prog" not in _CACHE:
        _CACHE["prog"] = _build_program()
    nc = _CACHE["prog"]

    res = bass_utils.run_bass_kernel_spmd(nc, in_maps, core_ids=list(range(NCORE)))

    h_full = np.empty((N, H), np.float32)
    inv = np.empty(SH, np.int64)
    inv[phys_local] = jloc
    for c in range(NCORE):
        h_full[c * SH + inv] = res.results[c]["hout"]

    z = h_full[:bs] @ W1.T + b1
    out = 1.0 / (1.0 + np.exp(-(z @ W2.T + b2)))
    return out.astype(np.float32)
